# revision 42
# baseline (speedup 1.0000x reference)
# GRU decoder kernel for Trainium2 (Bass/Tile), data-parallel over batch.
#
# Problem (per reference):
#   h0 = tanh(latent @ Wd + bd)                      [B, H]
#   x  = latent @ W + b[0]; xz, xr, xh = split(x, 3) [B, 3H]
#   for t in range(T):   (reset_after GRU, recurrent bias b[1])
#       rec = h @ U + b[1]; rz, rr, rh = split(rec, 3)
#       z = sigmoid(xz + rz); r = sigmoid(xr + rr)
#       hh = tanh(xh + r * rh)
#       h = z*h + (1-z)*hh        -> out[:, t, :]
#
# Sharding: batch 1024 -> 8 cores x 128 rows. Weights replicated. The T loop
# runs locally per core; no collectives.
#
# Per-core per-step dataflow (layout [batch=partitions, features=free]):
#   PE   : per gate g in (r, h, z): identity-matmul accumulates the x-side
#          bias/projection into that gate's own PSUM bank, then 4 K-chunk
#          matmuls of h_T @ U.  float32r operands (1 cycle/row vs 4 for f32).
#          z is issued last: it is consumed late, so its matmuls fill PE idle
#          time during the tail.
#   ACT  : r = sigmoid(ps_r), z = sigmoid(ps_z), zc = sigmoid(-ps_z) [= 1-z],
#          hh = tanh(t2), half the hT copies.
#   DVE  : t1 = r*ps_h, t2 = t1+xh, d = zc*hh, h_new = c1+d (4x128 chunks),
#          half the hT copies.
#   GP   : c1 = z*h
#   PE   : per-128-chunk transpose h_new -> own PSUM bank -> SBUF hT_j copy.
#   DMA  : h_new -> out[:, t, :]
#
# Separate tiles per gate-PSUM / per hT chunk keep Tile's dependency tracking
# fine-grained (a reader only waits for its own producer, not the whole
# 15-matmul burst).

import numpy as np

B, LD, H, T_DEF = 1024, 256, 512, 128
H3 = 3 * H
NCORES = 8
BS = B // NCORES  # 128 batch rows per core

_BUILD_CACHE = {}


def _build(T, tail_chunks=2):
    import concourse.bass as bass
    import concourse.mybir as mybir
    import concourse.tile as tile
    from concourse import bacc
    from concourse.masks import make_identity

    f32 = mybir.dt.float32
    f32r = mybir.dt.float32r
    AF = mybir.ActivationFunctionType
    OP = mybir.AluOpType

    nc = bacc.Bacc(None, target_bir_lowering=False, debug=False)

    latT = nc.dram_tensor("latT", [LD, BS], f32, kind="ExternalInput")
    wd_d = nc.dram_tensor("wd", [LD, H], f32, kind="ExternalInput")
    w_d = nc.dram_tensor("w", [LD, H3], f32, kind="ExternalInput")
    u_d = nc.dram_tensor("u", [H, H3], f32r, kind="ExternalInput")
    # bx = b[0] with b[1] folded into the z/r thirds; bh = b[1] h-third
    bx_d = nc.dram_tensor("bx", [H3], f32, kind="ExternalInput")
    bh_d = nc.dram_tensor("bh", [H], f32r, kind="ExternalInput")
    bd_d = nc.dram_tensor("bd", [H], f32, kind="ExternalInput")
    out_d = nc.dram_tensor("out", [BS, T, H], f32, kind="ExternalOutput")

    # gate column ranges in the 3H axis (reference order: z, r, h)
    ZS, RS, HS = slice(0, H), slice(H, 2 * H), slice(2 * H, H3)

    with tile.TileContext(nc) as tc:
        with (
            tc.tile_pool(name="singles", bufs=1) as singles,
            tc.tile_pool(name="work", bufs=3) as work,
            tc.tile_pool(name="hpool", bufs=3) as hpool,
            tc.tile_pool(name="htpool", bufs=3) as htpool,
            tc.tile_pool(name="psg", bufs=1, space="PSUM") as psg,
            tc.tile_pool(name="pst", bufs=1, space="PSUM") as pst,
        ):
            # ---- load constants -------------------------------------------
            lat = [singles.tile([128, BS], f32, tag=f"lat{j}", name=f"lat{j}")
                   for j in range(2)]
            for j in range(2):
                nc.sync.dma_start(out=lat[j], in_=latT[128 * j : 128 * (j + 1), :])
            wd = [singles.tile([128, H], f32, tag=f"wd{j}", name=f"wd{j}")
                  for j in range(2)]
            for j in range(2):
                nc.sync.dma_start(out=wd[j], in_=wd_d[128 * j : 128 * (j + 1), :])
            w = [singles.tile([128, H3], f32, tag=f"w{j}", name=f"w{j}")
                 for j in range(2)]
            for j in range(2):
                nc.sync.dma_start(out=w[j], in_=w_d[128 * j : 128 * (j + 1), :])
            u = [singles.tile([128, H3], f32r, tag=f"u{k}", name=f"u{k}")
                 for k in range(4)]
            for k in range(4):
                nc.sync.dma_start(out=u[k], in_=u_d[128 * k : 128 * (k + 1), :])

            def bcast(handle, n):
                ap = handle[:]
                return bass.AP(tensor=ap.tensor, offset=ap.offset,
                               ap=[[0, 128], [1, n]])

            xbias = singles.tile([128, H3], f32, tag="xbias")
            nc.gpsimd.dma_start(out=xbias, in_=bcast(bx_d, H3))
            b1h = singles.tile([128, H], f32r, tag="b1h")
            nc.gpsimd.dma_start(out=b1h, in_=bcast(bh_d, H))
            bdt = singles.tile([128, H], f32, tag="bdt")
            nc.gpsimd.dma_start(out=bdt, in_=bcast(bd_d, H))

            ident = singles.tile([128, 128], f32, tag="ident")
            make_identity(nc, ident)
            identr = singles.tile([128, 128], f32r, tag="identr")
            nc.scalar.copy(identr, ident)

            # PSUM tiles: 3 gate banks + 4 transpose banks + 1 prologue = 8
            ps_z = psg.tile([128, H], f32, tag="ps_z")
            ps_r = psg.tile([128, H], f32, tag="ps_r")
            ps_h = psg.tile([128, H], f32, tag="ps_h")
            gate_ps = [ps_z, ps_r, ps_h]
            tp = [pst.tile([128, 128], f32, tag=f"tp{j}", name=f"tp{j}")
                  for j in range(4)]
            pd = pst.tile([128, H], f32, tag="pd")

            # ---- prologue: h0 and x-projection (full fp32 precision) ------
            nc.tensor.matmul(pd, ident, bdt, start=True, stop=False)
            nc.tensor.matmul(pd, lat[0], wd[0], start=False, stop=False)
            nc.tensor.matmul(pd, lat[1], wd[1], start=False, stop=True)
            h = hpool.tile([128, H], f32, tag="h")
            nc.scalar.activation(h, pd, AF.Tanh)

            for gi, s in ((0, ZS), (1, RS), (2, HS)):
                px = gate_ps[gi]
                nc.tensor.matmul(px, ident, xbias[:, s], start=True, stop=False)
                nc.tensor.matmul(px, lat[0], w[0][:, s], start=False, stop=False)
                nc.tensor.matmul(px, lat[1], w[1][:, s], start=False, stop=True)
            # x-projection: rounded f32r copy for matmul use + fp32 copy of xh
            xzr = singles.tile([128, 2 * H], f32r, tag="xzr")
            nc.scalar.copy(xzr[:, ZS], gate_ps[0])
            nc.scalar.copy(xzr[:, RS], gate_ps[1])
            xh32 = singles.tile([128, H], f32, tag="xh32")
            nc.scalar.copy(xh32, gate_ps[2])

            hT = [htpool.tile([128, 128], f32r, tag=f"hT{j}", name=f"hT{j}")
                  for j in range(4)]
            for j in range(4):
                cs = slice(128 * j, 128 * (j + 1))
                nc.tensor.transpose(tp[j], h[:, cs], ident)
                nc.scalar.copy(hT[j], tp[j])

            # ---- steady-state T loop --------------------------------------
            nch = H // tail_chunks
            for t in range(T):
                # gate matmul bursts; r first (needed earliest), h last
                ps_z = psg.tile([128, H], f32, tag="ps_z")
                ps_r = psg.tile([128, H], f32, tag="ps_r")
                ps_h = psg.tile([128, H], f32, tag="ps_h")
                # r and h bursts gate the tail; z matmuls are issued after
                # (they run in PE idle time during the tail -- z is only
                # consumed by zc/c1/d, late in the step)
                for ps, s, xsrc in ((ps_r, RS, xzr[:, RS]), (ps_h, HS, b1h),
                                    (ps_z, ZS, xzr[:, ZS])):
                    nc.tensor.matmul(ps, identr, xsrc, start=True, stop=False)
                    for k in range(4):
                        nc.tensor.matmul(ps, hT[k], u[k][:, s],
                                         start=False, stop=(k == 3))

                r = work.tile([128, H], f32, tag="r")
                nc.scalar.activation(r, ps_r, AF.Sigmoid)
                z = work.tile([128, H], f32, tag="z")
                nc.scalar.activation(z, ps_z, AF.Sigmoid)
                # zc = 1-z via sigmoid(-pre_z) on ACT; c1 = z*h on GPSIMD --
                # both off the DVE critical chain
                zc = work.tile([128, H], f32, tag="zc")
                nc.scalar.activation(zc, ps_z, AF.Sigmoid, scale=-1.0)
                c1 = work.tile([128, H], f32, tag="c1")
                for lo in range(0, H, 128):
                    nc.gpsimd.tensor_mul(c1[:, lo:lo+128], z[:, lo:lo+128],
                                         h[:, lo:lo+128])

                # h_new = c1 + zc*hh, chunked so chunk 0's hT copies unblock
                # the next burst's k=0/1 matmuls early
                chunks = [(0, 128), (128, 256), (256, 384), (384, H)]
                t1 = work.tile([128, H], f32, tag="t1")
                t2 = work.tile([128, H], f32, tag="t2")
                hh = work.tile([128, H], f32, tag="hh")
                d = work.tile([128, H], f32, tag="d")
                hnew = hpool.tile([128, H], f32, tag="h")
                hT_new = [htpool.tile([128, 128], f32r, tag=f"hT{j}",
                                      name=f"hTn{j}") for j in range(4)]
                tpn = [pst.tile([128, 128], f32, tag=f"tp{j}", name=f"tpn{j}")
                       for j in range(4)]
                for lo, hi in chunks:
                    cs = slice(lo, hi)
                    nc.vector.tensor_mul(t1[:, cs], r[:, cs], ps_h[:, cs])
                    nc.vector.tensor_add(t2[:, cs], t1[:, cs], xh32[:, cs])
                    nc.scalar.activation(hh[:, cs], t2[:, cs], AF.Tanh)
                    nc.vector.tensor_mul(d[:, cs], zc[:, cs], hh[:, cs])
                    nc.vector.tensor_add(hnew[:, cs], c1[:, cs], d[:, cs])
                    for j in range(lo // 128, hi // 128):
                        js = slice(128 * j, 128 * (j + 1))
                        nc.tensor.transpose(tpn[j], hnew[:, js], ident)
                        # alternate copy engine so the 4 copies pairwise overlap
                        if j % 2 == 0:
                            nc.scalar.copy(hT_new[j], tpn[j])
                        else:
                            nc.vector.tensor_copy(hT_new[j], tpn[j])

                nc.sync.dma_start(out=out_d[:, t, :], in_=hnew)
                h = hnew
                hT = hT_new

    nc.compile()
    return nc


def kernel(latent, Wd, bd, W, U, b, T, _trace=False):
    from concourse.bass_utils import run_bass_kernel_spmd

    latent = np.ascontiguousarray(np.asarray(latent, dtype=np.float32))
    Wd = np.ascontiguousarray(np.asarray(Wd, dtype=np.float32))
    bd = np.ascontiguousarray(np.asarray(bd, dtype=np.float32))
    W = np.ascontiguousarray(np.asarray(W, dtype=np.float32))
    U = np.ascontiguousarray(np.asarray(U, dtype=np.float32))
    b = np.ascontiguousarray(np.asarray(b, dtype=np.float32))
    T = int(T)

    key = (T,)
    if key not in _BUILD_CACHE:
        _BUILD_CACHE[key] = _build(T)
    nc = _BUILD_CACHE[key]

    bx = b[0].copy()
    bx[: 2 * H] += b[1][: 2 * H]
    bh = np.ascontiguousarray(b[1][2 * H :])

    in_maps = []
    for c in range(NCORES):
        rows = slice(c * BS, (c + 1) * BS)
        in_maps.append({
            "latT": np.ascontiguousarray(latent[rows].T),
            "wd": Wd, "w": W, "u": U,
            "bx": bx, "bh": bh, "bd": bd,
        })

    res = run_bass_kernel_spmd(nc, in_maps, core_ids=list(range(NCORES)),
                               trace=_trace)
    if _trace and res.exec_time_ns is not None:
        print(f"HW exec time: {res.exec_time_ns} ns")
        if res.instructions_and_trace is not None:
            print(f"trace: {res.instructions_and_trace[1]}")

    out = np.concatenate([r["out"] for r in res.results], axis=0)
    return out
